# revision 32
# baseline (speedup 1.0000x reference)
"""Trainium2 Bass kernel for causal multi-head attention block (nn_Attention).

B1 variant (fallback): zero-collective, K/V computed fully per core,
with the multi-queue col-blocked DMA prologue.

Sharding: 8 cores = 4 batches x 2 interleaved query-stripe halves.
Core (b, half) owns query stripes {128*(2i+half) : i=0..7} of batch b.
Each core computes K/V for the full sequence of its batch (zero-collective
SPMD), Q only for its own 1024 queries, causally-tapered attention, and the
final FC for its own output rows.
"""

import numpy as np
import ml_dtypes

B = 4
S = 2048
EMB = 1024
HEADS = 16
DH = 64
NCORES = 8
NSTRIPE = 8          # query stripes per core
QW = NSTRIPE * 128   # queries per core
KV_TILES = 16        # 2048 / 128

BF16 = ml_dtypes.bfloat16

_compiled = None


def _build():
    from concourse import bacc, tile, mybir

    nc = bacc.Bacc("TRN2", target_bir_lowering=False, debug=False,
                   num_devices=NCORES)
    f32 = mybir.dt.float32
    bf16 = mybir.dt.bfloat16
    Exp = mybir.ActivationFunctionType.Exp
    Mult = mybir.AluOpType.mult
    Add = mybir.AluOpType.add

    xT_kv = nc.dram_tensor("xT_kv", [EMB, S], bf16, kind="ExternalInput")
    xT_q = nc.dram_tensor("xT_q", [EMB, QW], bf16, kind="ExternalInput")
    wqkv = nc.dram_tensor("wqkv", [EMB, 3 * EMB], bf16, kind="ExternalInput")
    wfc = nc.dram_tensor("wfc", [EMB, EMB], bf16, kind="ExternalInput")
    bqkv_t = nc.dram_tensor("bqkv_t", [128, 24], f32, kind="ExternalInput")
    bv_row = nc.dram_tensor("bv_row", [1, EMB], f32, kind="ExternalInput")
    bfc_row = nc.dram_tensor("bfc_row", [1, EMB], f32, kind="ExternalInput")
    masks = nc.dram_tensor("masks", [128, 512], bf16, kind="ExternalInput")
    out = nc.dram_tensor("out", [QW, EMB], f32, kind="ExternalOutput")

    # attention run structure: per pair, 3 uniform (scores, PV) runs
    RUNS = [
        (0, list(range(8))),       # q cols [0:512),   kv tiles 0..7 (diag)
        (1, list(range(8))),       # q cols [512:1024), kv tiles 0..7 (full)
        (1, list(range(8, 16))),   # q cols [512:1024), kv tiles 8..15 (diag)
    ]

    with tile.TileContext(nc) as tc:
        with (
            tc.tile_pool(name="consts", bufs=1) as consts,
            tc.tile_pool(name="vpool", bufs=1) as vpool,
            tc.tile_pool(name="xkvp", bufs=1) as xkvp,
            tc.tile_pool(name="xqp", bufs=1) as xqp,
            tc.tile_pool(name="ktpool", bufs=3) as ktpool,
            tc.tile_pool(name="qtpool", bufs=3) as qtpool,
            tc.tile_pool(name="wkp", bufs=2) as wkp,
            tc.tile_pool(name="attn", bufs=1) as attnp,
            tc.tile_pool(name="wfcp", bufs=1) as wfcp,
            tc.tile_pool(name="probs", bufs=10) as probsp,
            tc.tile_pool(name="numsb", bufs=2) as numsbp,
            tc.tile_pool(name="rbp", bufs=2) as rbp,
            tc.tile_pool(name="osb", bufs=2) as osbp,
            tc.tile_pool(name="pwork", bufs=3, space="PSUM") as pwork,
            tc.tile_pool(name="pnum", bufs=2, space="PSUM") as pnum,
        ):
            # ---- constants ----
            mask_sb = consts.tile([128, 512], bf16, tag="mask")
            nc.scalar.dma_start(out=mask_sb[:], in_=masks.ap()[:])
            bqkv_sb = consts.tile([128, 24], f32, tag="bqkv")
            nc.scalar.dma_start(out=bqkv_sb[:], in_=bqkv_t.ap()[:])
            bv_bc = consts.tile([128, EMB], f32, tag="bvbc")
            bfc_bc = consts.tile([128, EMB], f32, tag="bfcbc")
            with tc.tile_pool(name="brows", bufs=1) as brows:
                bv_sb = brows.tile([1, EMB], f32, tag="bvrow")
                nc.scalar.dma_start(out=bv_sb[:], in_=bv_row.ap()[:])
                bfc_sb = brows.tile([1, EMB], f32, tag="bfcrow")
                nc.scalar.dma_start(out=bfc_sb[:], in_=bfc_row.ap()[:])
                nc.gpsimd.partition_broadcast(bv_bc[:], bv_sb[:], channels=128)
                nc.gpsimd.partition_broadcast(bfc_bc[:], bfc_sb[:],
                                              channels=128)

            # input DMAs are spread across queues so no single queue
            # serializes the prologue: xkv on sync (col-blocked so the
            # first K-proj chunk depends on only 1MB), wk0/wq0 + wv on
            # scalar, xq + wfc on gpsimd.
            wfc_sb = [wfcp.tile([128, EMB], bf16, tag=f"wf{e}", name=f"wf{e}")
                      for e in range(8)]

            xkv = [xkvp.tile([128, S], bf16, tag=f"xk{e}", name=f"xk{e}")
                   for e in range(8)]
            xq = [xqp.tile([128, QW], bf16, tag=f"xq{e}", name=f"xq{e}")
                  for e in range(8)]

            v_sb = [vpool.tile([128, HEADS, DH + 1], bf16, tag=f"v{st}",
                               name=f"v{st}") for st in range(KV_TILES)]
            attn_sb = [attnp.tile([128, QW], bf16, tag=f"at{p}", name=f"at{p}")
                       for p in range(NSTRIPE)]

            # ---- pipelined K/Q projection + attention ----
            kq_state = {}

            def proj_pair(p):
                """DMA the pair's W columns and alloc its tiles."""
                wk = [wkp.tile([128, 128], bf16, tag=f"wk{e}",
                               name=f"wk{e}_{p}") for e in range(8)]
                wq = [wkp.tile([128, 128], bf16, tag=f"wq{e}",
                               name=f"wq{e}_{p}") for e in range(8)]
                eng = nc.scalar if p == 0 else nc.sync
                for e in range(8):
                    eng.dma_start(
                        out=wk[e][:],
                        in_=wqkv.ap()[128 * e:128 * e + 128,
                                      1024 + 128 * p:1024 + 128 * p + 128])
                    eng.dma_start(
                        out=wq[e][:],
                        in_=wqkv.ap()[128 * e:128 * e + 128,
                                      128 * p:128 * p + 128])
                kt = ktpool.tile([128, S], bf16, tag="kt", name=f"kt{p}")
                qt = qtpool.tile([128, QW], bf16, tag="qt", name=f"qt{p}")
                kq_state[p] = (wk, wq, kt, qt)

            def proj_piece(p, ci, sub, ps, eh=None):
                """One projection e-chain (or an e-half of it if eh given)."""
                wk, wq, kt, qt = kq_state[p]
                erange = range(8) if eh is None else range(4 * eh, 4 * eh + 4)
                if ci < 2:
                    sc = 2 * ci + sub
                    for e in erange:
                        nc.tensor.matmul(
                            ps[:, 512 * sub:512 * sub + 512],
                            lhsT=wk[e][:],
                            rhs=xkv[e][:, 512 * sc:512 * sc + 512],
                            start=(e == 0), stop=(e == 7),
                        )
                else:
                    for e in erange:
                        nc.tensor.matmul(
                            ps[:, 512 * sub:512 * sub + 512],
                            lhsT=wq[e][:],
                            rhs=xq[e][:, 512 * sub:512 * sub + 512],
                            start=(e == 0), stop=(e == 7),
                        )

            def proj_finish(p, ci, ps):
                wk, wq, kt, qt = kq_state[p]
                if ci < 2:
                    nc.vector.tensor_scalar_add(
                        kt[:, 1024 * ci:1024 * ci + 1024], ps[:],
                        bqkv_sb[:, 8 + p:9 + p],
                    )
                else:
                    nc.vector.tensor_scalar_add(
                        qt[:], ps[:], bqkv_sb[:, p:p + 1],
                    )

            def proj_chunk(p, ci):
                ps = pwork.tile([128, 1024], f32, tag="pw", name="pw")
                proj_piece(p, ci, 0, ps)
                proj_piece(p, ci, 1, ps)
                proj_finish(p, ci, ps)

            proj_pair(0)

            def v_chain(st):
                nc.vector.memset(v_sb[st][:, :, DH:DH + 1], 1.0)
                ps = pwork.tile([128, 1024], f32, tag="pw", name="pw")
                for dc in range(2):
                    for e in range(8):
                        nc.tensor.matmul(
                            ps[:, 512 * dc:512 * dc + 512],
                            lhsT=xkv[e][:, 128 * st:128 * st + 128],
                            rhs=wv[e][:, 512 * dc:512 * dc + 512],
                            start=(e == 0), stop=(e == 7),
                        )
                nc.vector.tensor_tensor(
                    out=v_sb[st][:, :, 0:DH],
                    in0=ps[:],
                    in1=bv_bc[:],
                    op=Add,
                )

            with tc.tile_pool(name="wvp", bufs=1) as wvp:
                wv = [wvp.tile([128, EMB], bf16, tag=f"wv{e}", name=f"wv{e}")
                      for e in range(8)]
                # xkv left half, col-blocked: the first K-proj chunk only
                # depends on the first 1MB.
                for e in range(8):
                    nc.sync.dma_start(
                        out=xkv[e][:, 0:512],
                        in_=xT_kv.ap()[128 * e:128 * e + 128, 0:512])
                for e in range(8):
                    nc.sync.dma_start(
                        out=xkv[e][:, 512:1024],
                        in_=xT_kv.ap()[128 * e:128 * e + 128, 512:1024])
                for e in range(8):
                    nc.scalar.dma_start(
                        out=wv[e][:],
                        in_=wqkv.ap()[128 * e:128 * e + 128, 2048:3072])
                for e in range(8):
                    nc.sync.dma_start(
                        out=xkv[e][:, 1024:2048],
                        in_=xT_kv.ap()[128 * e:128 * e + 128, 1024:2048])
                for e in range(8):
                    nc.gpsimd.dma_start(
                        out=xq[e][:],
                        in_=xT_q.ap()[128 * e:128 * e + 128, :])
                for e in range(8):
                    nc.gpsimd.dma_start(
                        out=wfc_sb[e][:],
                        in_=wfc.ap()[128 * e:128 * e + 128, :])
                # compute emission: K chunk 0 first (smallest dep set),
                # V chains + remaining K/Q chunks fill in as DMAs land.
                proj_chunk(0, 0)
                for st in range(8):
                    v_chain(st)
                proj_chunk(0, 1)
                for st in range(8, 16):
                    v_chain(st)
                proj_chunk(0, 2)

            def normalize(p, qh, nums):
                # attn^T = num^T[0:64] * (1/num^T[64]) for one query half
                for hh in range(2):
                    rr = numsbp.tile([DH + 1, 512], f32, tag="ns",
                                     name="ns")
                    nc.vector.tensor_copy(
                        rr[DH:DH + 1, :], nums[hh][DH:DH + 1, :])
                    r0 = rbp.tile([1, 512], f32, tag="r0", name="r0")
                    nc.sync.dma_start(out=r0[:], in_=rr[DH:DH + 1, :])
                    nc.vector.reciprocal_approx_fast(out=r0[:], in_=r0[:])
                    rb = rbp.tile([DH, 512], f32, tag="rb", name="rb")
                    nc.gpsimd.partition_broadcast(rb[:], r0[:],
                                                  channels=DH)
                    nc.vector.tensor_tensor(
                        out=attn_sb[p][64 * hh:64 * hh + 64,
                                       512 * qh:512 * qh + 512],
                        in0=nums[hh][0:DH, :], in1=rb[:], op=Mult,
                    )

            # FC group helpers (a group = one output q-chunk; its e=0..6
            # matmuls need only attn tiles of pairs 0..6, so groups qc=0,1
            # interleave into pair-7's runs as stall filler)
            def fc_finish(qc, ps_fc):
                for cc in range(2):
                    nc.tensor.matmul(
                        ps_fc[:, 512 * cc:512 * cc + 512],
                        lhsT=attn_sb[7][:, 128 * qc:128 * qc + 128],
                        rhs=wfc_sb[7][:, 512 * cc:512 * cc + 512],
                        start=False, stop=True,
                    )
                osb = osbp.tile([128, EMB], f32, tag="ot", name="ot")
                nc.vector.tensor_tensor(
                    out=osb[:], in0=ps_fc[:], in1=bfc_bc[:], op=Add,
                )
                nc.sync.dma_start(
                    out=out.ap()[128 * qc:128 * qc + 128, :], in_=osb[:])

            FC_PIECES = [(e, cc) for e in range(7) for cc in range(2)]
            fc_state = {}

            for p in range(8):
                _, _, kt, qt = kq_state[p]
                if p + 1 < 8:
                    proj_pair(p + 1)
                nums_by_qh = {}
                for ri, (qh, ks) in enumerate(RUNS):
                    qlo, qhi = 512 * qh, 512 * qh + 512
                    if ri in (0, 1):
                        nums_by_qh[qh] = [
                            pnum.tile([DH + 1, 512], f32, tag="pn",
                                      name="pn") for _ in range(2)]
                    nums = nums_by_qh[qh]
                    probs = {}
                    ps_proj = (pwork.tile([128, 1024], f32, tag="pw",
                                          name="pw") if p + 1 < 8 else None)
                    if p == 7 and ri >= 1:
                        fc_finish(ri - 1, fc_state.pop(ri - 1))
                    if p == 7:
                        fc_state[ri] = pwork.tile([128, 1024], f32,
                                                  tag="pw", name="pw")
                    # --- score + exp run (both heads share one psum tile) ---
                    for ki, k in enumerate(ks):
                        a = max(qlo, 128 * (k // 2))
                        n = qhi - a
                        off = a - qlo
                        ps = pwork.tile([128, 1024], f32, tag="pw",
                                        name="pw")
                        for hh in range(2):
                            lo, hi = 64 * hh, 64 * hh + 64
                            nc.tensor.matmul(
                                ps[:, 512 * hh + off:512 * hh + 512],
                                lhsT=kt[lo:hi, 128 * k:128 * k + 128],
                                rhs=qt[lo:hi, a:qhi],
                                start=True, stop=True,
                            )
                        pr = probsp.tile([128, 1024], bf16, tag="pr",
                                         name="pr")
                        probs[k] = pr
                        ps3 = ps[:].rearrange("p (a b) -> p a b", a=2)
                        pr3 = pr[:].rearrange("p (a b) -> p a b", a=2)
                        nc.scalar.activation(
                            pr3[:, :, off:512], ps3[:, :, off:512], Exp,
                            scale=0.125,
                        )
                        if a == 128 * (k // 2):
                            nc.vector.tensor_tensor(
                                out=pr3[:, :, off:off + 128],
                                in0=pr3[:, :, off:off + 128],
                                in1=mask_sb[:, 256 * (k % 2):
                                            256 * (k % 2) + 256],
                                op=Mult,
                            )
                        # next pair's projection chains fill the exp drain
                        if ps_proj is not None and ki in (1, 3, 5, 7):
                            proj_piece(p + 1, ri, (ki - 1) // 4,
                                       ps_proj, eh=((ki - 1) // 2) % 2)
                        # pair 7: FC chains (e<=6) fill the drain instead
                        if p == 7:
                            for e, cc in FC_PIECES[2 * ki:2 * ki + 2]:
                                nc.tensor.matmul(
                                    fc_state[ri][:, 512 * cc:512 * cc + 512],
                                    lhsT=attn_sb[e][:, 128 * ri:
                                                    128 * ri + 128],
                                    rhs=wfc_sb[e][:, 512 * cc:512 * cc + 512],
                                    start=(e == 0), stop=False,
                                )
                    if ps_proj is not None:
                        proj_finish(p + 1, ri, ps_proj)
                    # --- PV run ---
                    for k in ks:
                        a = max(qlo, 128 * (k // 2))
                        off = a - qlo
                        for hh in range(2):
                            h = 2 * p + hh
                            nc.tensor.matmul(
                                nums[hh][:, off:512],
                                lhsT=v_sb[k][:, h, 0:DH + 1],
                                rhs=probs[k][:, 512 * hh + off:
                                             512 * hh + 512],
                                start=(ri in (0, 1) and k == ks[0]),
                                stop=(ri == 2 and k == ks[-1]) or
                                     (ri == 0 and k == ks[-1]),
                            )
                    if ri in (0, 2):
                        normalize(p, qh, nums)
                del kq_state[p]

            # ---- FC ----
            fc_finish(2, fc_state.pop(2))
            for qc in range(3, 8):
                osb = osbp.tile([128, EMB], f32, tag="ot", name="ot")
                ps = pwork.tile([128, 1024], f32, tag="pw", name="pw")
                for cc in range(2):
                    for e in range(8):
                        nc.tensor.matmul(
                            ps[:, 512 * cc:512 * cc + 512],
                            lhsT=attn_sb[e][:, 128 * qc:128 * qc + 128],
                            rhs=wfc_sb[e][:, 512 * cc:512 * cc + 512],
                            start=(e == 0), stop=(e == 7),
                        )
                nc.vector.tensor_tensor(
                    out=osb[:], in0=ps[:], in1=bfc_bc[:], op=Add,
                )
                nc.sync.dma_start(
                    out=out.ap()[128 * qc:128 * qc + 128, :],
                    in_=osb[:])

    nc.compile()
    return nc


def _get_compiled():
    global _compiled
    if _compiled is None:
        _compiled = _build()
    return _compiled


def _make_in_maps(x, w_qkv, b_qkv, w_fc, b_fc):
    wqkv_bf = w_qkv.astype(BF16)
    wfc_bf = w_fc.astype(BF16)
    bqkv_t = np.ascontiguousarray(b_qkv.reshape(24, 128).T.astype(np.float32))
    bv_row = np.ascontiguousarray(
        b_qkv[2 * EMB:3 * EMB].astype(np.float32))[None, :]
    bfc_row = np.ascontiguousarray(b_fc.astype(np.float32))[None, :]

    tri = np.tril(np.ones((128, 128), dtype=np.float32)).T  # valid kv<=q
    zeros = np.zeros((128, 128), np.float32)
    ones = np.ones((128, 128), np.float32)
    mask_by_half = {
        0: np.concatenate([tri, tri, zeros, zeros], axis=1),
        1: np.concatenate([ones, ones, tri, tri], axis=1),
    }

    in_maps = []
    for core in range(NCORES):
        b, half = core // 2, core % 2
        xT = x[b].T.astype(BF16)                      # [EMB, S]
        cols = np.concatenate(
            [np.arange(128 * (2 * i + half), 128 * (2 * i + half) + 128)
             for i in range(NSTRIPE)])
        in_maps.append({
            "xT_kv": np.ascontiguousarray(xT),
            "xT_q": np.ascontiguousarray(xT[:, cols]),
            "wqkv": wqkv_bf,
            "wfc": wfc_bf,
            "bqkv_t": bqkv_t,
            "bv_row": bv_row,
            "bfc_row": bfc_row,
            "masks": mask_by_half[half].astype(BF16),
        })
    return in_maps


def kernel(x, w_qkv, b_qkv, w_fc, b_fc, _trace=False, _trace_cores=None):
    from concourse import bass_utils
    from concourse.bass_interp import get_hw_module

    x = np.asarray(x, dtype=np.float32)
    w_qkv = np.asarray(w_qkv, dtype=np.float32)
    b_qkv = np.asarray(b_qkv, dtype=np.float32)
    w_fc = np.asarray(w_fc, dtype=np.float32)
    b_fc = np.asarray(b_fc, dtype=np.float32)

    nc = _get_compiled()
    in_maps = _make_in_maps(x, w_qkv, b_qkv, w_fc, b_fc)

    old_m = nc.m
    nc.m = get_hw_module(nc.m)
    try:
        res = bass_utils.run_bass_kernel_spmd(
            nc, in_maps, core_ids=list(range(NCORES)), trace=_trace,
            trace_cores=_trace_cores)
    finally:
        nc.m = old_m

    y = np.empty((B, S, EMB), dtype=np.float32)
    for core in range(NCORES):
        b, half = core // 2, core % 2
        o = res.results[core]["out"]
        for i in range(NSTRIPE):
            g = 2 * i + half
            y[b, 128 * g:128 * g + 128, :] = o[128 * i:128 * i + 128, :]
    if _trace:
        kernel._last_exec_time_ns = res.exec_time_ns
        kernel._last_results = res
    return y
